# revision 14
# baseline (speedup 1.0000x reference)
"""AttentionBlock (GroupNorm + single-head self-attention + residual) on 8 TRN2 cores.

Strategy: pure data-parallel over batch (16 items -> 2 per core), no collectives.
All big matmuls run in fp8(e4m3) with perf_mode=DoubleRow (two K=128 subtiles
per instruction -> ~1.5x PE throughput vs bf16). Scale bookkeeping (all exact
powers of two, folded into PSUM evictions):
  - weights stored as 32*W in fp8 (w std 0.044 -> 1.4, clear of subnormals)
  - hn (GroupNorm out), q, k, v stored at natural ~N(0,1) scale in fp8
  - exp tiles stored as exp(logit)/16 (activation bias -ln16) to keep the
    fp8 range safe; softmax denominators are computed from the same fp8
    values (DoubleRow ones-matmuls accumulating in PSUM across the S phase)
    so normalization is exactly consistent; ones value 1/8 -> recip = 8/sum.
  - PV out evicted as ps * (8/sum) -> ~N(0,1.5) fp8
  - proj psum = (32wp)@(8*attn) = 256*proj; the residual is added IN PSUM via
    a (256*I) bf16 matmul over x_bf16; eviction = ACT * 2^-8 + bpp -> f32 out,
    where bpp = wp@bv + bp (the V bias commutes: sum_j attn[i,j] = 1).
DMA: per-queue order matters (~16 HW engines per queue, but in-order queues
and per-partition-line packetization). x tiles (2KB lines) go FIRST, one per
queue (sync/scalar/gpsimd/vector); the small per-channel vectors are packed
into the sel tensor (one 592B-line DMA) instead of five 16B-line DMAs; the
fp8 weights follow their consumers' needs; id256 loads late on the vector
queue. Outputs alternate sync/vector.
Engine balance per batch: PE ~37us (DR-MMs + ones/residual/sel), ACT ~17us
(q/k/exp/proj evictions), DVE ~22us (stats, hn apply, V/ou evictions, recip),
gpsimd only the partition_broadcast. GroupNorm rsqrt stays on DVE (fast-inv-
sqrt + 2 Newton steps) so the ACT table never leaves the exp set.
"""

import numpy as np
import ml_dtypes

B_TOT, C, H, W = 16, 512, 32, 32
N = H * W            # 1024
NCORES = 8
BPC = B_TOT // NCORES  # 2 batch items per core
CT = C // 128        # 4 channel tiles
NT = N // 128        # 8 position tiles
NCH = N // 512       # 2 free-dim chunks of 512
GS = 16              # group size (channels per group)
EPS = 1e-5
SCALE = float(C) ** -0.5
LN16 = 2.772588722239781

_CACHE = {}


def _build_bass():
    import concourse.bass as bass  # noqa: F401
    import concourse.tile as tile
    from concourse import bacc, mybir

    F32 = mybir.dt.float32
    BF16 = mybir.dt.bfloat16
    FP8 = mybir.dt.float8e4
    U8 = mybir.dt.uint8
    Alu = mybir.AluOpType
    Act = mybir.ActivationFunctionType
    DR = mybir.MatmulPerfMode.DoubleRow

    nc = bacc.Bacc("TRN2", target_bir_lowering=False, debug=False,
                   num_devices=NCORES)

    x_ext = nc.dram_tensor("x", [BPC, 128, CT, N], BF16, kind="ExternalInput").ap()
    w_ext = {
        name: nc.dram_tensor(name, [128, CT, 512], FP8, kind="ExternalInput").ap()
        for name in ("wq", "wk", "wv", "wp")
    }
    # sel matrix (cols 0:128) + packed per-channel vectors (cols 128:148):
    # gamma, beta, bq, bk, bpp at 128 + i*CT
    sv_ext = nc.dram_tensor("selvec", [128, 128 + 5 * CT], F32,
                            kind="ExternalInput").ap()
    id_ext = nc.dram_tensor("id256", [128, 128], BF16, kind="ExternalInput").ap()
    out_ext = nc.dram_tensor("out", [BPC, 128, CT, N], F32, kind="ExternalOutput").ap()

    with tile.TileContext(nc) as tc:
        with (
            tc.tile_pool(name="consts", bufs=1) as consts,
            tc.tile_pool(name="xp", bufs=2) as xp,
            tc.tile_pool(name="hnp", bufs=2) as hnp,
            tc.tile_pool(name="qkp", bufs=1) as qkp,
            tc.tile_pool(name="vp", bufs=1) as vp,
            tc.tile_pool(name="ep", bufs=1) as ep,
            tc.tile_pool(name="oup", bufs=1) as oup,
            tc.tile_pool(name="outp", bufs=3) as outp,
            tc.tile_pool(name="rp", bufs=1) as rp,
            tc.tile_pool(name="smallp", bufs=8) as smallp,
            tc.tile_pool(name="psq", bufs=2, space="PSUM") as psq,
            tc.tile_pool(name="psv", bufs=2, space="PSUM") as psv,
            tc.tile_pool(name="psg", bufs=1, space="PSUM") as psg,
        ):
            # ---- device-built constants (no DMA) ----
            magic_sb = consts.tile([128, 1], mybir.dt.int32, tag="magic")
            nc.vector.memset(magic_sb[:], 0x5F3759DF)
            negln16_sb = consts.tile([128, 1], F32, tag="negln16")
            nc.vector.memset(negln16_sb[:], -LN16)
            # ones (value 1/8) for the softmax-sum matmuls
            ones_sb = consts.tile([128, 1], FP8, tag="ones")
            nc.vector.memset(ones_sb[:].bitcast(U8), 0x20)  # e4m3 0.125
            wu_sb = consts.tile([128, 512], BF16, tag="wu")
            nc.vector.memset(wu_sb[:], 0.0)

            # ---- DMA-loaded constants & x (queue order is the schedule) ----
            x_tiles = [[None] * CT for _ in range(BPC)]
            engs = [nc.sync, nc.scalar, nc.gpsimd, nc.sync]

            def load_x(b):
                for t in range(CT):
                    xt = xp.tile([128, N], BF16, tag=f"x{t}", name=f"x_b{b}_t{t}")
                    engs[t].dma_start(xt[:], x_ext[b, :, t, :])
                    x_tiles[b][t] = xt
                return x_tiles[b]

            x0 = load_x(0)

            sv_sb = consts.tile([128, 128 + 5 * CT], F32, tag="selvec")
            nc.sync.dma_start(sv_sb[:], sv_ext[:])
            sel_sb = sv_sb[:, 0:128]
            vec_sb = {
                name: sv_sb[:, 128 + i * CT:128 + (i + 1) * CT]
                for i, name in enumerate(("gamma", "beta", "bq", "bk", "bpp"))
            }

            w_sb = {}
            for name, eng in (("wq", nc.scalar), ("wv", nc.gpsimd),
                              ("wk", nc.sync), ("wp", nc.gpsimd)):
                w_sb[name] = consts.tile([128, CT, 512], FP8, tag=name,
                                         name=f"w_{name}")
                eng.dma_start(w_sb[name][:], w_ext[name][:])

            x1 = load_x(1)

            id_sb = consts.tile([128, 128], BF16, tag="id256")
            nc.gpsimd.dma_start(id_sb[:], id_ext[:])

            # PE warm-up: throwaway matmuls fill the initial DMA wait so the
            # HAM clock gate is already released (2.4 GHz) when the real
            # matmuls start (a >3.4us PE idle window re-throttles it). Split
            # in two groups so gn(0)'s sel-matmul slots in between.
            def warmup(n):
                ps_wu = psv.tile([128, 512], F32, tag="vmm", name="ps_warm")
                for i in range(n):
                    nc.tensor.matmul(ps_wu[:], wu_sb[:, 0:128], wu_sb[:],
                                     start=(i == 0), stop=(i == n - 1))
                nc.vector.tensor_copy(wu_sb[:, 0:4], ps_wu[:, 0:4])

            warmup(8)

            def gn(b, xts):
                # per-channel stats over n, group-combine via block-diagonal
                # selector matmul, rsqrt on DVE, apply as tensor_scalar -> fp8
                mv = smallp.tile([128, CT, 2], F32, tag="mv", name=f"mv{b}")
                for t in range(CT):
                    stats = smallp.tile([128, 2, 6], F32, tag="stats",
                                        name=f"st{b}_{t}")
                    nc.vector.bn_stats(stats[:, 0, :], xts[t][:, 0:512])
                    nc.vector.bn_stats(stats[:, 1, :], xts[t][:, 512:1024])
                    nc.vector.bn_aggr(mv[:, t, :], stats[:])
                # s_all[:, 0, t]=mean_t, s_all[:, 1, t]=E[x^2]_t
                s_all = smallp.tile([128, 2, CT], F32, tag="s_all", name=f"s{b}")
                nc.vector.tensor_copy(s_all[:, 0, :], mv[:, :, 0])
                nc.vector.tensor_tensor(s_all[:, 1, :], mv[:, :, 0], mv[:, :, 0],
                                        Alu.mult)
                nc.vector.tensor_tensor(s_all[:, 1, :], s_all[:, 1, :],
                                        mv[:, :, 1], Alu.add)
                # group-combine matmul into a regular rotation slot of psq
                gs = psq.tile([128, N], F32, tag="mm", name=f"gs{b}")
                nc.tensor.matmul(gs[:, 0:2 * CT], sel_sb, s_all[:],
                                 start=True, stop=True)
                gsb = smallp.tile([128, 2, CT], F32, tag="gsb", name=f"gb{b}")
                nc.vector.tensor_copy(gsb[:], gs[:, 0:2 * CT])
                ab = smallp.tile([128, 4, CT], F32, tag="ab", name=f"ab{b}")
                va = ab[:, 0, :]         # var
                vp_ = ab[:, 1, :]        # var + eps
                y = ab[:, 2, :]
                tmp = ab[:, 3, :]
                nc.vector.tensor_tensor(va, gsb[:, 0, :], gsb[:, 0, :], Alu.mult)
                nc.vector.tensor_tensor(va, gsb[:, 1, :], va, Alu.subtract)
                # rstd = rsqrt(var+eps) entirely on DVE (fast-inverse-sqrt seed
                # + 2 Newton steps) so the scalar engine's activation tables
                # never leave the exp set (table reloads are 2.7us each).
                nc.vector.tensor_scalar_add(vp_, va, EPS)
                I32 = mybir.dt.int32
                nc.vector.tensor_scalar(y.bitcast(I32), vp_.bitcast(I32), 1,
                                        None, Alu.arith_shift_right)
                nc.vector.tensor_tensor(y.bitcast(I32),
                                        magic_sb[:].to_broadcast([128, CT]),
                                        y.bitcast(I32), Alu.subtract)
                for _ in range(2):  # Newton: y *= 1.5 - 0.5*v*y^2
                    nc.vector.tensor_tensor(tmp, y, y, Alu.mult)
                    nc.vector.tensor_tensor(tmp, tmp, vp_, Alu.mult)
                    nc.vector.tensor_scalar(tmp, tmp, -0.5, 1.5, Alu.mult,
                                            Alu.add)
                    nc.vector.tensor_tensor(y, y, tmp, Alu.mult)
                a_all = ab[:, 0, :]      # reuse var slot: a = rstd*gamma
                bsh = ab[:, 3, :]
                nc.vector.tensor_tensor(a_all, y, vec_sb["gamma"], Alu.mult)
                nc.vector.tensor_tensor(bsh, gsb[:, 0, :], a_all, Alu.mult)
                nc.vector.tensor_tensor(bsh, vec_sb["beta"], bsh, Alu.subtract)
                hn_sb = hnp.tile([128, CT, N], FP8, tag="hn", name=f"hn{b}")
                for t in range(CT):
                    nc.vector.tensor_scalar(hn_sb[:, t, :], xts[t][:],
                                            ab[:, 0, t:t + 1], ab[:, 3, t:t + 1],
                                            Alu.mult, Alu.add)
                return hn_sb

            def qk(b, hn_sb):
                q_sb = qkp.tile([128, CT, N], FP8, tag="q", name=f"q{b}")
                k_sb = qkp.tile([128, CT, N], FP8, tag="k", name=f"k{b}")
                for dst, wname, bname in ((q_sb, "wq", "bq"), (k_sb, "wk", "bk")):
                    for t in range(CT):
                        ps = psq.tile([128, N], F32, tag="mm",
                                      name=f"ps{wname}{b}_{t}")
                        for ch in range(NCH):
                            cs = slice(ch * 512, (ch + 1) * 512)
                            for itp in range(CT // 2):
                                nc.tensor.matmul(
                                    ps[:, cs],
                                    w_sb[wname][:, 2 * itp:2 * itp + 2,
                                                t * 128:(t + 1) * 128],
                                    hn_sb[:, 2 * itp:2 * itp + 2, cs],
                                    start=(itp == 0), stop=(itp == CT // 2 - 1),
                                    perf_mode=DR)
                        nc.scalar.activation(dst[:, t, :], ps[:], Act.Identity,
                                             bias=vec_sb[bname][:, t:t + 1],
                                             scale=1.0 / 32)
                return q_sb, k_sb

            def v(b, hn_sb):
                # V computed TRANSPOSED: vT[n, c], evicted * 1/32 -> fp8
                vT_sb = vp.tile([128, NT, 512], FP8, tag="vT", name=f"vT{b}")
                for jt in range(NT):
                    ps = psv.tile([128, 512], F32, tag="vmm", name=f"psv{b}_{jt}")
                    for itp in range(CT // 2):
                        nc.tensor.matmul(
                            ps[:],
                            hn_sb[:, 2 * itp:2 * itp + 2, jt * 128:(jt + 1) * 128],
                            w_sb["wv"][:, 2 * itp:2 * itp + 2, :],
                            start=(itp == 0), stop=(itp == CT // 2 - 1),
                            perf_mode=DR)
                    nc.vector.tensor_scalar(vT_sb[:, jt, :], ps[:], 1.0 / 32,
                                            None, Alu.mult)
                return vT_sb

            def st_exp(b, q_sb, k_sb):
                # S^T tiles [j, i]; exp fused into eviction (scale 1/sqrt(c),
                # bias -ln16). Softmax sums accumulate in PSUM via DoubleRow
                # ones-MMs (value 1/8 -> recip = 8/sum), trailing the exp
                # evictions by a tile of slack.
                e_sb = ep.tile([128, NT, N], FP8, tag="e", name=f"e{b}")
                sums_ps = psg.tile([1, NCH, 512], F32, tag="sums",
                                   name=f"sums{b}")

                def ones_mm(jtp):
                    for jt in (2 * jtp, 2 * jtp + 1):
                        for ch in range(NCH):
                            cs = slice(ch * 512, (ch + 1) * 512)
                            nc.tensor.matmul(
                                sums_ps[:, ch, :], ones_sb[:],
                                e_sb[:, jt, cs],
                                start=(jt == 0), stop=(jt == NT - 1))

                done = 0
                for jt in range(NT):
                    ps = psq.tile([128, N], F32, tag="mm", name=f"pss{b}_{jt}")
                    for ch in range(NCH):
                        cs = slice(ch * 512, (ch + 1) * 512)
                        for ctp in range(CT // 2):
                            nc.tensor.matmul(
                                ps[:, cs],
                                k_sb[:, 2 * ctp:2 * ctp + 2,
                                     jt * 128:(jt + 1) * 128],
                                q_sb[:, 2 * ctp:2 * ctp + 2, cs],
                                start=(ctp == 0), stop=(ctp == CT // 2 - 1),
                                perf_mode=DR)
                    nc.scalar.activation(e_sb[:, jt, :], ps[:], Act.Exp,
                                         scale=SCALE, bias=negln16_sb[:])
                    # emit a trailing ones pair once both of its e tiles have
                    # been produced AND one more S tile is in flight (slack
                    # for the in-order ACT queue)
                    while 2 * (done + 1) + 1 < jt:
                        ones_mm(done)
                        done += 1
                while done < NT // 2:
                    ones_mm(done)
                    done += 1
                return e_sb, sums_ps

            def recip(b, sums_ps):
                sums_sb = rp.tile([1, N], F32, tag="sums", name=f"sm{b}")
                nc.vector.tensor_copy(sums_sb[:], sums_ps[:])
                sumb_sb = rp.tile([128, N], F32, tag="sumb", name=f"sb{b}")
                nc.gpsimd.partition_broadcast(sumb_sb[:], sums_sb[:])
                recip_sb = rp.tile([128, N], F32, tag="recip", name=f"rc{b}")
                nc.vector.reciprocal_approx_fast(recip_sb[:], sumb_sb[:])
                return recip_sb

            def pv(b, vT_sb, e_sb, recip_sb):
                ou_sb = oup.tile([128, CT, N], FP8, tag="ou", name=f"ou{b}")
                for ct in range(CT):
                    ps = psq.tile([128, N], F32, tag="mm", name=f"pso{b}_{ct}")
                    for ch in range(NCH):
                        cs = slice(ch * 512, (ch + 1) * 512)
                        for jtp in range(NT // 2):
                            nc.tensor.matmul(
                                ps[:, cs],
                                vT_sb[:, 2 * jtp:2 * jtp + 2,
                                      ct * 128:(ct + 1) * 128],
                                e_sb[:, 2 * jtp:2 * jtp + 2, cs],
                                start=(jtp == 0), stop=(jtp == NT // 2 - 1),
                                perf_mode=DR)
                    nc.vector.tensor_tensor(ou_sb[:, ct, :], ps[:], recip_sb[:],
                                            Alu.mult)
                return ou_sb

            def proj(b, ou_sb, xts):
                # psum = 256*(wp@attn) + 256*x (identity matmul); eviction on
                # ACT: * 2^-8 + bpp -> f32 out
                oeng = [nc.sync, nc.scalar, nc.sync, nc.scalar]
                for ot in range(CT):
                    ps = psq.tile([128, N], F32, tag="mm", name=f"psp{b}_{ot}")
                    for ch in range(NCH):
                        cs = slice(ch * 512, (ch + 1) * 512)
                        for ctp in range(CT // 2):
                            nc.tensor.matmul(
                                ps[:, cs],
                                w_sb["wp"][:, 2 * ctp:2 * ctp + 2,
                                           ot * 128:(ot + 1) * 128],
                                ou_sb[:, 2 * ctp:2 * ctp + 2, cs],
                                start=(ctp == 0), stop=False,
                                perf_mode=DR)
                        nc.tensor.matmul(ps[:, cs], id_sb[:], xts[ot][:, cs],
                                         start=False, stop=True)
                    o_sb = outp.tile([128, N], F32, tag="o", name=f"o{b}_{ot}")
                    nc.scalar.activation(o_sb[:], ps[:], Act.Identity,
                                         scale=1.0 / 256,
                                         bias=vec_sb["bpp"][:, ot:ot + 1])
                    oeng[ot].dma_start(out_ext[b, :, ot, :], o_sb[:])

            # ---- software pipeline over the two batch items ----
            h0 = gn(0, x0)
            warmup(8)
            q0, k0 = qk(0, h0)
            v0 = v(0, h0)
            h1 = gn(1, x1)
            e0, sp0 = st_exp(0, q0, k0)
            r0 = recip(0, sp0)
            q1, k1 = qk(1, h1)
            o0 = pv(0, v0, e0, r0)
            v1 = v(1, h1)
            e1, sp1 = st_exp(1, q1, k1)
            r1 = recip(1, sp1)
            proj(0, o0, x0)
            o1 = pv(1, v1, e1, r1)
            proj(1, o1, x1)

    nc.compile()
    return nc


def _prep_vec(v):
    # [C] f32 -> [128, CT] with v_sb[p, t] = v[t*128 + p]
    return np.ascontiguousarray(
        np.asarray(v, dtype=np.float32).reshape(CT, 128).T)


def _prep_w8(w):
    # [C, C] (out, in) -> lhsT layout [128, CT, 512] fp8 e4m3, scaled by 32:
    # w_sb[p, it, o] = 32 * w[o, it*128 + p]
    wT = np.asarray(w, dtype=np.float32).T * 32.0
    wT = np.clip(wT, -240.0, 240.0)
    return np.ascontiguousarray(
        wT.reshape(CT, 128, C).transpose(1, 0, 2).astype(ml_dtypes.float8_e4m3))


def _prep_host_inputs(x, gamma, beta, wq, bq, wk, bk, wv, bv, wp, bp):
    x = np.asarray(x, dtype=np.float32)
    # [16, C, H, W] -> [16, 128, CT, N] bf16
    xr = np.ascontiguousarray(
        x.reshape(B_TOT, CT, 128, N).transpose(0, 2, 1, 3)).astype(
            ml_dtypes.bfloat16)

    bpp = np.asarray(wp, np.float32) @ np.asarray(bv, np.float32) \
        + np.asarray(bp, np.float32)
    sel = np.kron(np.eye(128 // GS, dtype=np.float32),
                  np.full((GS, GS), 1.0 / GS, dtype=np.float32))
    selvec = np.concatenate(
        [sel] + [_prep_vec(v) for v in (gamma, beta, bq, bk, bpp)], axis=1)
    common = {
        "wq": _prep_w8(wq), "wk": _prep_w8(wk), "wv": _prep_w8(wv),
        "wp": _prep_w8(wp),
        "selvec": np.ascontiguousarray(selvec),
        "id256": np.ascontiguousarray(
            (np.eye(128, dtype=np.float32) * 256.0).astype(ml_dtypes.bfloat16)),
    }
    return xr, common


def kernel(x, gamma, beta, wq, bq, wk, bk, wv, bv, wp, bp):
    from concourse.bass_utils import run_bass_kernel_spmd

    nc = _CACHE.get("nc")
    if nc is None:
        nc = _CACHE["nc"] = _build_bass()

    xr, common = _prep_host_inputs(x, gamma, beta, wq, bq, wk, bk, wv, bv,
                                   wp, bp)
    in_maps = [
        {"x": np.ascontiguousarray(xr[c * BPC:(c + 1) * BPC]), **common}
        for c in range(NCORES)
    ]
    res = run_bass_kernel_spmd(nc, in_maps, core_ids=list(range(NCORES)))
    # [BPC, 128, CT, N] per core -> [16, C, H, W]
    out = np.concatenate([r["out"] for r in res.results], axis=0)
    return np.ascontiguousarray(
        out.transpose(0, 2, 1, 3)).reshape(B_TOT, C, H, W)


# revision 19
# speedup vs baseline: 1.1358x; 1.1358x over previous
"""AttentionBlock (GroupNorm + single-head self-attention + residual) on 8 TRN2 cores.

Strategy: pure data-parallel over batch (16 items -> 2 per core), no collectives.
All big matmuls run in fp8(e4m3) with perf_mode=DoubleRow (two K=128 subtiles
per instruction -> ~1.5x PE throughput vs bf16). Scale bookkeeping (all exact
powers of two, folded into PSUM evictions):
  - weights stored as 32*W in fp8 (w std 0.044 -> 1.4, clear of subnormals)
  - hn (GroupNorm out), q, k, v stored at natural ~N(0,1) scale in fp8
  - exp tiles stored as exp(logit)/16 (activation bias -ln16) to keep the
    fp8 range safe; softmax denominators are computed from the same fp8
    values (DoubleRow ones-matmuls accumulating in PSUM across the S phase)
    so normalization is exactly consistent; ones value 1/8 -> recip = 8/sum.
  - PV out evicted as ps * (8/sum) -> ~N(0,1.5) fp8
  - proj psum = (32wp)@(8*attn) = 256*proj; the residual is added IN PSUM via
    a (256*I) bf16 matmul over x_bf16; eviction = ACT * 2^-8 + bpp -> f32 out,
    where bpp = wp@bv + bp (the V bias commutes: sum_j attn[i,j] = 1).
DMA: per-queue order matters (~16 HW engines per queue, but in-order queues
and per-partition-line packetization). x tiles (2KB lines) go FIRST, one per
queue (sync/scalar/gpsimd/vector); the small per-channel vectors are packed
into the sel tensor (one 592B-line DMA) instead of five 16B-line DMAs; the
fp8 weights follow their consumers' needs; id256 loads late on the vector
queue. Outputs alternate sync/vector.
Engine balance per batch: PE ~37us (DR-MMs + ones/residual/sel), ACT ~17us
(q/k/exp/proj evictions), DVE ~22us (stats, hn apply, V/ou evictions, recip),
gpsimd only the partition_broadcast. GroupNorm rsqrt stays on DVE (fast-inv-
sqrt + 2 Newton steps) so the ACT table never leaves the exp set.
"""

import numpy as np
import ml_dtypes

B_TOT, C, H, W = 16, 512, 32, 32
N = H * W            # 1024
NCORES = 8
BPC = B_TOT // NCORES  # 2 batch items per core
CT = C // 128        # 4 channel tiles
NT = N // 128        # 8 position tiles
NCH = N // 512       # 2 free-dim chunks of 512
GS = 16              # group size (channels per group)
EPS = 1e-5
SCALE = float(C) ** -0.5
LN16 = 2.772588722239781

_CACHE = {}


def _build_bass():
    import concourse.bass as bass  # noqa: F401
    import concourse.tile as tile
    from concourse import bacc, mybir

    F32 = mybir.dt.float32
    BF16 = mybir.dt.bfloat16
    FP8 = mybir.dt.float8e4
    U8 = mybir.dt.uint8
    Alu = mybir.AluOpType
    Act = mybir.ActivationFunctionType
    DR = mybir.MatmulPerfMode.DoubleRow

    nc = bacc.Bacc("TRN2", target_bir_lowering=False, debug=False,
                   num_devices=NCORES)

    x_ext = nc.dram_tensor("x", [BPC, 128, CT, N], BF16, kind="ExternalInput").ap()
    w_ext = {
        name: nc.dram_tensor(name, [128, CT, 512], FP8, kind="ExternalInput").ap()
        for name in ("wq", "wk", "wv", "wp")
    }
    # sel matrix (cols 0:128) + packed per-channel vectors (cols 128:148):
    # gamma, beta, bq, bk, bpp at 128 + i*CT
    sv_ext = nc.dram_tensor("selvec", [128, 128 + 5 * CT], F32,
                            kind="ExternalInput").ap()
    id_ext = nc.dram_tensor("id256", [128, 128], BF16, kind="ExternalInput").ap()
    out_ext = nc.dram_tensor("out", [BPC, 128, CT, N], F32, kind="ExternalOutput").ap()

    with tile.TileContext(nc) as tc:
        with (
            tc.tile_pool(name="consts", bufs=1) as consts,
            tc.tile_pool(name="xp", bufs=2) as xp,
            tc.tile_pool(name="hnp", bufs=2) as hnp,
            tc.tile_pool(name="qkp", bufs=1) as qkp,
            tc.tile_pool(name="vp", bufs=1) as vp,
            tc.tile_pool(name="ep", bufs=1) as ep,
            tc.tile_pool(name="oup", bufs=1) as oup,
            tc.tile_pool(name="outp", bufs=3) as outp,
            tc.tile_pool(name="rp", bufs=1) as rp,
            tc.tile_pool(name="smallp", bufs=8) as smallp,
            tc.tile_pool(name="psq", bufs=2, space="PSUM") as psq,
            tc.tile_pool(name="psv", bufs=2, space="PSUM") as psv,
            tc.tile_pool(name="psg", bufs=1, space="PSUM") as psg,
        ):
            # ---- device-built constants (no DMA) ----
            magic_sb = consts.tile([128, 1], mybir.dt.int32, tag="magic")
            nc.vector.memset(magic_sb[:], 0x5F3759DF)
            negln16_sb = consts.tile([128, 1], F32, tag="negln16")
            nc.vector.memset(negln16_sb[:], -LN16)
            # ones (value 1/8) for the softmax-sum matmuls
            ones_sb = consts.tile([128, 1], FP8, tag="ones")
            nc.vector.memset(ones_sb[:].bitcast(U8), 0x20)  # e4m3 0.125
            wu_sb = consts.tile([128, 512], BF16, tag="wu")
            nc.vector.memset(wu_sb[:], 0.0)

            # ---- DMA-loaded constants & x (queue order is the schedule) ----
            x_tiles = [[None] * CT for _ in range(BPC)]
            engs = [nc.sync, nc.scalar, nc.gpsimd, nc.sync]

            def load_x(b):
                for t in range(CT):
                    xt = xp.tile([128, N], BF16, tag=f"x{t}", name=f"x_b{b}_t{t}")
                    engs[t].dma_start(xt[:], x_ext[b, :, t, :])
                    x_tiles[b][t] = xt
                return x_tiles[b]

            x0 = load_x(0)

            # selvec is a 128-line small-line transfer (line cost dominates,
            # not bytes): split by partition thirds across the three queues
            sv_sb = consts.tile([128, 128 + 5 * CT], F32, tag="selvec")
            for (lo, hi), eng in (((0, 43), nc.sync), ((43, 86), nc.scalar),
                                  ((86, 128), nc.gpsimd)):
                eng.dma_start(sv_sb[lo:hi, :], sv_ext[lo:hi, :])
            sel_sb = sv_sb[:, 0:128]
            vec_sb = {
                name: sv_sb[:, 128 + i * CT:128 + (i + 1) * CT]
                for i, name in enumerate(("gamma", "beta", "bq", "bk", "bpp"))
            }

            w_sb = {}
            for name, eng in (("wq", nc.scalar), ("wv", nc.gpsimd),
                              ("wk", nc.sync), ("wp", nc.gpsimd)):
                w_sb[name] = consts.tile([128, CT, 512], FP8, tag=name,
                                         name=f"w_{name}")
                eng.dma_start(w_sb[name][:], w_ext[name][:])

            x1 = load_x(1)

            id_sb = consts.tile([128, 128], BF16, tag="id256")
            nc.gpsimd.dma_start(id_sb[:], id_ext[:])

            # PE warm-up: throwaway matmuls fill the initial DMA wait so the
            # HAM clock gate is already released (2.4 GHz) when the real
            # matmuls start (a >3.4us PE idle window re-throttles it). Split
            # in two groups so gn(0)'s sel-matmul slots in between.
            def warmup(n):
                ps_wu = psv.tile([128, 512], F32, tag="vmm", name="ps_warm")
                for i in range(n):
                    nc.tensor.matmul(ps_wu[:], wu_sb[:, 0:128], wu_sb[:],
                                     start=(i == 0), stop=(i == n - 1))
                nc.vector.tensor_copy(wu_sb[:, 0:4], ps_wu[:, 0:4])

            warmup(8)

            def gn(b, xts):
                # per-channel stats over n, group-combine via block-diagonal
                # selector matmul, rsqrt on DVE (fast-inv-sqrt + 1 Newton step
                # -- ~0.2% max scale error, far below the fp8 quantization
                # noise; keeps the scalar engine's table on the exp set),
                # apply as scale/shift -> fp8, alternating ACT/DVE per tile.
                # s_all[:, 0, t]=mean_t, s_all[:, 1, t]=var_t (aggr writes the
                # strided slices directly), then var slot -> E[x^2].
                s_all = smallp.tile([128, 2, CT], F32, tag="s_all", name=f"s{b}")
                for t in range(CT):
                    stats = smallp.tile([128, 2, 6], F32, tag="stats",
                                        name=f"st{b}_{t}")
                    nc.vector.bn_stats(stats[:, 0, :], xts[t][:, 0:512])
                    nc.vector.bn_stats(stats[:, 1, :], xts[t][:, 512:1024])
                    nc.vector.bn_aggr(s_all[:, :, t], stats[:])
                mn2 = smallp.tile([128, CT], F32, tag="mn2", name=f"mn2{b}")
                nc.vector.tensor_tensor(mn2[:], s_all[:, 0, :], s_all[:, 0, :],
                                        Alu.mult)
                nc.vector.tensor_tensor(s_all[:, 1, :], s_all[:, 1, :], mn2[:],
                                        Alu.add)
                # group-combine matmul into a regular rotation slot of psq
                gs = psq.tile([128, N], F32, tag="mm", name=f"gs{b}")
                nc.tensor.matmul(gs[:, 0:2 * CT], sel_sb, s_all[:],
                                 start=True, stop=True)
                gsb = smallp.tile([128, 2, CT], F32, tag="gsb", name=f"gb{b}")
                nc.vector.tensor_copy(gsb[:], gs[:, 0:2 * CT])
                ab = smallp.tile([128, 4, CT], F32, tag="ab", name=f"ab{b}")
                va = ab[:, 0, :]         # var
                vp_ = ab[:, 1, :]        # var + eps
                y = ab[:, 2, :]
                tmp = ab[:, 3, :]
                nc.vector.tensor_tensor(va, gsb[:, 0, :], gsb[:, 0, :], Alu.mult)
                nc.vector.tensor_tensor(va, gsb[:, 1, :], va, Alu.subtract)
                nc.vector.tensor_scalar_add(vp_, va, EPS)
                I32 = mybir.dt.int32
                nc.vector.tensor_scalar(y.bitcast(I32), vp_.bitcast(I32), 1,
                                        None, Alu.arith_shift_right)
                nc.vector.tensor_tensor(y.bitcast(I32),
                                        magic_sb[:].to_broadcast([128, CT]),
                                        y.bitcast(I32), Alu.subtract)
                for _ in range(1):  # Newton: y *= 1.5 - 0.5*v*y^2
                    nc.vector.tensor_tensor(tmp, y, y, Alu.mult)
                    nc.vector.tensor_tensor(tmp, tmp, vp_, Alu.mult)
                    nc.vector.tensor_scalar(tmp, tmp, -0.5, 1.5, Alu.mult,
                                            Alu.add)
                    nc.vector.tensor_tensor(y, y, tmp, Alu.mult)
                a_all = ab[:, 0, :]      # reuse var slot: a = rstd*gamma
                bsh = ab[:, 3, :]
                nc.vector.tensor_tensor(a_all, y, vec_sb["gamma"], Alu.mult)
                nc.vector.tensor_tensor(bsh, gsb[:, 0, :], a_all, Alu.mult)
                nc.vector.tensor_tensor(bsh, vec_sb["beta"], bsh, Alu.subtract)
                hn_sb = hnp.tile([128, CT, N], FP8, tag="hn", name=f"hn{b}")
                for t in range(CT):
                    if t % 2 == 0:
                        nc.scalar.activation(hn_sb[:, t, :], xts[t][:],
                                             Act.Identity,
                                             bias=ab[:, 3, t:t + 1],
                                             scale=ab[:, 0, t:t + 1])
                    else:
                        nc.vector.tensor_scalar(hn_sb[:, t, :], xts[t][:],
                                                ab[:, 0, t:t + 1],
                                                ab[:, 3, t:t + 1],
                                                Alu.mult, Alu.add)
                return hn_sb

            def qk(b, hn_sb):
                q_sb = qkp.tile([128, CT, N], FP8, tag="q", name=f"q{b}")
                k_sb = qkp.tile([128, CT, N], FP8, tag="k", name=f"k{b}")
                for dst, wname, bname in ((q_sb, "wq", "bq"), (k_sb, "wk", "bk")):
                    for t in range(CT):
                        ps = psq.tile([128, N], F32, tag="mm",
                                      name=f"ps{wname}{b}_{t}")
                        for ch in range(NCH):
                            cs = slice(ch * 512, (ch + 1) * 512)
                            for itp in range(CT // 2):
                                nc.tensor.matmul(
                                    ps[:, cs],
                                    w_sb[wname][:, 2 * itp:2 * itp + 2,
                                                t * 128:(t + 1) * 128],
                                    hn_sb[:, 2 * itp:2 * itp + 2, cs],
                                    start=(itp == 0), stop=(itp == CT // 2 - 1),
                                    perf_mode=DR)
                        # evictions alternate ACT/DVE so neither engine paces
                        # the 4-DR-MM psum fill
                        if t % 2 == 0:
                            nc.scalar.activation(dst[:, t, :], ps[:],
                                                 Act.Identity,
                                                 bias=vec_sb[bname][:, t:t + 1],
                                                 scale=1.0 / 32)
                        else:
                            nc.vector.tensor_scalar(dst[:, t, :], ps[:],
                                                    1.0 / 32,
                                                    vec_sb[bname][:, t:t + 1],
                                                    Alu.mult, Alu.add)
                return q_sb, k_sb

            def v(b, hn_sb):
                # V computed TRANSPOSED: vT[n, c], evicted * 1/32 -> fp8
                vT_sb = vp.tile([128, NT, 512], FP8, tag="vT", name=f"vT{b}")
                for jt in range(NT):
                    ps = psv.tile([128, 512], F32, tag="vmm", name=f"psv{b}_{jt}")
                    for itp in range(CT // 2):
                        nc.tensor.matmul(
                            ps[:],
                            hn_sb[:, 2 * itp:2 * itp + 2, jt * 128:(jt + 1) * 128],
                            w_sb["wv"][:, 2 * itp:2 * itp + 2, :],
                            start=(itp == 0), stop=(itp == CT // 2 - 1),
                            perf_mode=DR)
                    nc.vector.tensor_scalar(vT_sb[:, jt, :], ps[:], 1.0 / 32,
                                            None, Alu.mult)
                return vT_sb

            def st_exp(b, q_sb, k_sb):
                # S^T tiles [j, i]; exp fused into eviction (scale 1/sqrt(c),
                # bias -ln16). Softmax sums accumulate in PSUM via DoubleRow
                # ones-MMs (value 1/8 -> recip = 8/sum), trailing the exp
                # evictions by a tile of slack.
                e_sb = ep.tile([128, NT, N], FP8, tag="e", name=f"e{b}")
                sums_ps = psg.tile([1, NCH, 512], F32, tag="sums",
                                   name=f"sums{b}")

                def ones_mm(jtp):
                    for jt in (2 * jtp, 2 * jtp + 1):
                        for ch in range(NCH):
                            cs = slice(ch * 512, (ch + 1) * 512)
                            nc.tensor.matmul(
                                sums_ps[:, ch, :], ones_sb[:],
                                e_sb[:, jt, cs],
                                start=(jt == 0), stop=(jt == NT - 1))

                done = 0
                for jt in range(NT):
                    ps = psq.tile([128, N], F32, tag="mm", name=f"pss{b}_{jt}")
                    for ch in range(NCH):
                        cs = slice(ch * 512, (ch + 1) * 512)
                        for ctp in range(CT // 2):
                            nc.tensor.matmul(
                                ps[:, cs],
                                k_sb[:, 2 * ctp:2 * ctp + 2,
                                     jt * 128:(jt + 1) * 128],
                                q_sb[:, 2 * ctp:2 * ctp + 2, cs],
                                start=(ctp == 0), stop=(ctp == CT // 2 - 1),
                                perf_mode=DR)
                    nc.scalar.activation(e_sb[:, jt, :], ps[:], Act.Exp,
                                         scale=SCALE, bias=negln16_sb[:])
                    # emit a trailing ones pair once both of its e tiles have
                    # been produced AND one more S tile is in flight (slack
                    # for the in-order ACT queue)
                    while 2 * (done + 1) + 1 < jt:
                        ones_mm(done)
                        done += 1
                while done < NT // 2:
                    ones_mm(done)
                    done += 1
                return e_sb, sums_ps

            def recip(b, sums_ps):
                sums_sb = rp.tile([1, N], F32, tag="sums", name=f"sm{b}")
                nc.vector.tensor_copy(sums_sb[:], sums_ps[:])
                sumb_sb = rp.tile([128, N], F32, tag="sumb", name=f"sb{b}")
                nc.gpsimd.partition_broadcast(sumb_sb[:], sums_sb[:])
                recip_sb = rp.tile([128, N], F32, tag="recip", name=f"rc{b}")
                nc.vector.reciprocal_approx_fast(recip_sb[:], sumb_sb[:])
                return recip_sb

            def pv(b, vT_sb, e_sb, recip_sb):
                ou_sb = oup.tile([128, CT, N], FP8, tag="ou", name=f"ou{b}")
                for ct in range(CT):
                    ps = psq.tile([128, N], F32, tag="mm", name=f"pso{b}_{ct}")
                    for ch in range(NCH):
                        cs = slice(ch * 512, (ch + 1) * 512)
                        for jtp in range(NT // 2):
                            nc.tensor.matmul(
                                ps[:, cs],
                                vT_sb[:, 2 * jtp:2 * jtp + 2,
                                      ct * 128:(ct + 1) * 128],
                                e_sb[:, 2 * jtp:2 * jtp + 2, cs],
                                start=(jtp == 0), stop=(jtp == NT // 2 - 1),
                                perf_mode=DR)
                    nc.vector.tensor_tensor(ou_sb[:, ct, :], ps[:], recip_sb[:],
                                            Alu.mult)
                return ou_sb

            def proj(b, ou_sb, xts):
                # psum = 256*(wp@attn) + 256*x (identity matmul); eviction on
                # ACT: * 2^-8 + bpp -> f32 out
                oeng = [nc.sync, nc.scalar, nc.sync, nc.scalar]
                for ot in range(CT):
                    ps = psq.tile([128, N], F32, tag="mm", name=f"psp{b}_{ot}")
                    for ch in range(NCH):
                        cs = slice(ch * 512, (ch + 1) * 512)
                        for ctp in range(CT // 2):
                            nc.tensor.matmul(
                                ps[:, cs],
                                w_sb["wp"][:, 2 * ctp:2 * ctp + 2,
                                           ot * 128:(ot + 1) * 128],
                                ou_sb[:, 2 * ctp:2 * ctp + 2, cs],
                                start=(ctp == 0), stop=False,
                                perf_mode=DR)
                        nc.tensor.matmul(ps[:, cs], id_sb[:], xts[ot][:, cs],
                                         start=False, stop=True)
                    o_sb = outp.tile([128, N], F32, tag="o", name=f"o{b}_{ot}")
                    if ot % 2 == 0:
                        nc.scalar.activation(o_sb[:], ps[:], Act.Identity,
                                             scale=1.0 / 256,
                                             bias=vec_sb["bpp"][:, ot:ot + 1])
                    else:
                        nc.vector.tensor_scalar(o_sb[:], ps[:], 1.0 / 256,
                                                vec_sb["bpp"][:, ot:ot + 1],
                                                Alu.mult, Alu.add)
                    oeng[ot].dma_start(out_ext[b, :, ot, :], o_sb[:])

            # ---- software pipeline over the two batch items ----
            h0 = gn(0, x0)
            warmup(8)
            warmup(8)
            q0, k0 = qk(0, h0)
            v0 = v(0, h0)
            h1 = gn(1, x1)
            e0, sp0 = st_exp(0, q0, k0)
            r0 = recip(0, sp0)
            q1, k1 = qk(1, h1)
            o0 = pv(0, v0, e0, r0)
            v1 = v(1, h1)
            e1, sp1 = st_exp(1, q1, k1)
            r1 = recip(1, sp1)
            proj(0, o0, x0)
            o1 = pv(1, v1, e1, r1)
            proj(1, o1, x1)

    nc.compile()
    return nc


def _prep_vec(v):
    # [C] f32 -> [128, CT] with v_sb[p, t] = v[t*128 + p]
    return np.ascontiguousarray(
        np.asarray(v, dtype=np.float32).reshape(CT, 128).T)


def _prep_w8(w):
    # [C, C] (out, in) -> lhsT layout [128, CT, 512] fp8 e4m3, scaled by 32:
    # w_sb[p, it, o] = 32 * w[o, it*128 + p]
    wT = np.asarray(w, dtype=np.float32).T * 32.0
    wT = np.clip(wT, -240.0, 240.0)
    return np.ascontiguousarray(
        wT.reshape(CT, 128, C).transpose(1, 0, 2).astype(ml_dtypes.float8_e4m3))


def _prep_host_inputs(x, gamma, beta, wq, bq, wk, bk, wv, bv, wp, bp):
    x = np.asarray(x, dtype=np.float32)
    # [16, C, H, W] -> [16, 128, CT, N] bf16
    xr = np.ascontiguousarray(
        x.reshape(B_TOT, CT, 128, N).transpose(0, 2, 1, 3)).astype(
            ml_dtypes.bfloat16)

    bpp = np.asarray(wp, np.float32) @ np.asarray(bv, np.float32) \
        + np.asarray(bp, np.float32)
    sel = np.kron(np.eye(128 // GS, dtype=np.float32),
                  np.full((GS, GS), 1.0 / GS, dtype=np.float32))
    selvec = np.concatenate(
        [sel] + [_prep_vec(v) for v in (gamma, beta, bq, bk, bpp)], axis=1)
    common = {
        "wq": _prep_w8(wq), "wk": _prep_w8(wk), "wv": _prep_w8(wv),
        "wp": _prep_w8(wp),
        "selvec": np.ascontiguousarray(selvec),
        "id256": np.ascontiguousarray(
            (np.eye(128, dtype=np.float32) * 256.0).astype(ml_dtypes.bfloat16)),
    }
    return xr, common


def kernel(x, gamma, beta, wq, bq, wk, bk, wv, bv, wp, bp):
    from concourse.bass_utils import run_bass_kernel_spmd

    nc = _CACHE.get("nc")
    if nc is None:
        nc = _CACHE["nc"] = _build_bass()

    xr, common = _prep_host_inputs(x, gamma, beta, wq, bq, wk, bk, wv, bv,
                                   wp, bp)
    in_maps = [
        {"x": np.ascontiguousarray(xr[c * BPC:(c + 1) * BPC]), **common}
        for c in range(NCORES)
    ]
    res = run_bass_kernel_spmd(nc, in_maps, core_ids=list(range(NCORES)))
    # [BPC, 128, CT, N] per core -> [16, C, H, W]
    out = np.concatenate([r["out"] for r in res.results], axis=0)
    return np.ascontiguousarray(
        out.transpose(0, 2, 1, 3)).reshape(B_TOT, C, H, W)
